# revision 1
# baseline (speedup 1.0000x reference)
"""Trainium2 Bass kernel for MultiHeadSelfAttention (RoPE + causal softmax).

Problem (hardcoded):
  x: (2, 2048, 512) f32, w_qkv: (1536, 512) f32, w_o: (512, 512) f32
  D_MODEL=512, N_HEADS=16, HEAD_DIM=32, ROPE_BASE=10000, causal.

Sharding: tensor-parallel over heads. Core c owns heads (2c, 2c+1) for both
batches. Each core computes its heads' q/k/v projections from the full x,
attention, and a Megatron-style row-parallel partial of the output
projection (out @ w_o.T restricted to its 64 input features). The host sums
the 8 partials (the row-parallel "unshard").

On-core layout highlights:
  - x is passed pre-transposed (xT [512, 4096]) so the d_model contraction
    sits on the partition axis for both projection orientations.
  - q,k are produced transposed ([feat, row]) and RoPE'd in that layout
    (rotate_half done with a block-diag permutation matmul on the PE).
  - scores are computed transposed (S.T [keys, queries]) so softmax'd P
    feeds the av matmul as weights without any transpose.
  - causal mask applied by accumulating -240*max(0, k-q) into the scores
    PSUM via a rank-128 A.T@B matmul of constant triangular matrices.
  - exp on the scalar engine (PSUM -> bf16 SBUF), with 1/sqrt(hd) folded
    into the activation scale. No max-subtraction (scores are provably
    small for this problem's scale).
  - row sums for softmax come from an extra all-ones column appended to v.
  - all big matmuls run float32r (1 cycle/row when N>=256) or bf16.
"""

import sys
import math
from contextlib import ExitStack

sys.path.insert(0, "/opt/trn_rl_repo")

import numpy as np
import ml_dtypes

import concourse.bass as bass
import concourse.tile as tile
from concourse import bacc, mybir
from concourse.bass_utils import run_bass_kernel_spmd

F32 = mybir.dt.float32
F32R = mybir.dt.float32r
BF16 = mybir.dt.bfloat16
EXP = mybir.ActivationFunctionType.Exp

B = 2
T = 2048
D = 512
NH = 16
HD = 32
NCORES = 8
R = B * T            # 4096 rows, row = b*T + t
NHL = NH // NCORES   # 2 heads per core
KC = T // 128        # 16 key chunks per batch
SCALE = 1.0 / math.sqrt(HD)
MASK_VAL = -240.0

def _bcast_free(ap_2d, n_inner):
    """[P, n] -> [P, n, n_inner] AP with the inner dim broadcast (step 0)."""
    return bass.AP(
        tensor=ap_2d.tensor,
        offset=ap_2d.offset,
        ap=list(ap_2d.ap[:-1]) + [list(ap_2d.ap[-1]), [0, n_inner]],
    )


def _emit(tc, io, loop_k=1):
    nc = tc.nc
    with ExitStack() as ctx:
        cpool = ctx.enter_context(tc.tile_pool(name="consts", bufs=1))
        mpool = ctx.enter_context(tc.tile_pool(name="main", bufs=1))
        spool = ctx.enter_context(tc.tile_pool(name="small", bufs=3))
        ppool = ctx.enter_context(tc.tile_pool(name="pk", bufs=2))
        # single PSUM pool, tags shared across phases (8 banks total):
        #   tagA [128,1024] x2 = 4 banks  (qk-proj / scores)
        #   tagB [128,512]  x2 = 2 banks  (shift/vT/vtr/atps/outps)
        #   tagC [128,8,33] x2 = 2 banks  (av accumulator groups)
        psum = ctx.enter_context(tc.tile_pool(name="psum", bufs=1, space="PSUM"))

        def tile_a():
            return psum.tile([128, 1024], F32, tag="A", bufs=2, name="psA")

        def tile_b(p=128, w=512):
            return psum.tile([p, w], F32, tag="B", bufs=2, name="psB")

        def tile_c():
            return psum.tile([128, 8, HD + 1], F32, tag="C", bufs=2, name="psC")

        # ---- constants (batched DMAs, spread over issue queues) ----
        wo = cpool.tile([64, 512], BF16, tag="wo")
        nc.scalar.dma_start(out=wo, in_=io["woT"])
        cmix = cpool.tile([128, 384], F32R, tag="cmix")
        nc.gpsimd.dma_start(out=cmix, in_=io["consts1"])
        permt = cmix[:, 0:128]
        trilA = cmix[:, 128:256]
        trilB = cmix[:, 256:384]
        ident = cpool.tile([128, 128], F32, tag="ident")
        nc.gpsimd.dma_start(out=ident, in_=io["ident"])
        wqkv = []
        for dc in range(4):
            w_t = cpool.tile([128, 192], F32R, tag=f"wqkv{dc}")
            nc.gpsimd.dma_start(out=w_t, in_=io["wqkvT"][dc * 128:(dc + 1) * 128, :])
            wqkv.append(w_t)

        # ---- persistent activations ----
        qkr = mpool.tile([128, R], F32R, tag="qkr")          # RoPE'd qT/kT
        ka = mpool.tile([64, R], F32R, tag="ka")             # k-half, base-aligned
        vall = mpool.tile([128, R // 128, NHL, HD + 1], BF16, tag="vall")
        ao = mpool.tile([128, B, KC, NHL, HD], BF16, tag="ao")  # attnout natural
        aoT = mpool.tile([64, R], BF16, tag="aoT")          # attnout transposed
        cosw = mpool.tile([128, T], F32, tag="cosw")        # one batch (shared)
        sinw = mpool.tile([128, T], F32, tag="sinw")

        nc.vector.memset(vall[:, :, :, HD:HD + 1], 1.0)     # softmax-sum column
        identb = cpool.tile([128, 128], BF16, tag="identb")
        nc.vector.tensor_copy(identb, ident)                # bf16 identity

        def emit_proj(bb):
            xt = [mpool.tile([128, T], F32R, tag=f"xt{dc}", bufs=1,
                             name=f"xt{dc}") for dc in range(4)]
            for j in range(4):
                for dc in range(4):
                    nc.sync.dma_start(
                        out=xt[dc][:, j * 512:(j + 1) * 512],
                        in_=io["xT"][dc * 128:(dc + 1) * 128,
                                     bb * T + j * 512:bb * T + (j + 1) * 512],
                    )
            if bb == 0:
                # after xt(b0) so x never queues behind these on the DMA rings
                nc.scalar.dma_start(out=cosw, in_=io["cosw"])
                nc.scalar.dma_start(out=sinw, in_=io["sinw"])

            for jl in range(4):
                colb = slice(jl * 512, (jl + 1) * 512)          # batch-local
                cols = slice(bb * T + jl * 512, bb * T + (jl + 1) * 512)
                # qT/kT projection: [feat, row] = wqkT.T @ xT
                qk_ps = tile_b()
                for dc in range(4):
                    nc.tensor.matmul(
                        qk_ps, wqkv[dc][:, 0:128], xt[dc][:, colb],
                        start=(dc == 0), stop=(dc == 3),
                    )
                # rotate_half via block-diag permutation (needs SBUF copy);
                # batch 0's copies ride the still-idle scalar engine
                qks = spool.tile([128, 512], F32R, tag="qks")
                if bb == 0:
                    nc.scalar.copy(qks, qk_ps)
                else:
                    nc.vector.tensor_copy(qks, qk_ps)
                sh_ps = tile_b()
                nc.tensor.matmul(sh_ps, permt, qks, start=True,
                                 stop=True)
                # qkr = qk*cos + shifted*sin_signed
                t1 = spool.tile([128, 512], F32, tag="t1")
                nc.vector.tensor_mul(t1, sh_ps, sinw[:, colb])
                nc.vector.tensor_mul(qkr[:, cols], qk_ps, cosw[:, colb])
                nc.vector.tensor_add(qkr[:, cols], qkr[:, cols], t1)
                # partition-aligned copy of the k rows (matmul requires lhsT
                # and rhs to share a base partition)
                nc.gpsimd.tensor_copy(ka[:, cols], qkr[64:128, cols])

                # vT projection: [feat, row]
                vt_ps = tile_b(64)
                for dc in range(4):
                    nc.tensor.matmul(
                        vt_ps, wqkv[dc][:, 128:192], xt[dc][:, colb],
                        start=(dc == 0), stop=(dc == 3),
                    )
                vt_sb = spool.tile([64, 512], BF16, tag="vtsb")
                if bb == 0:
                    nc.scalar.copy(vt_sb, vt_ps)
                else:
                    nc.vector.tensor_copy(vt_sb, vt_ps)
                # transpose v back to natural [row, feat] (bf16 on copy-out);
                # 4 transposes share one PSUM bank (disjoint 64-col regions)
                vtr_ps = psum.tile([128, 256], BF16, tag="B", bufs=2,
                                   name="psBv")
                for jj in range(4):
                    nc.tensor.transpose(
                        vtr_ps[:, jj * 64:(jj + 1) * 64],
                        vt_sb[:, jj * 128:(jj + 1) * 128],
                        identb[0:64, 0:64],
                    )
                for jj in range(4):
                    nc.vector.tensor_copy(
                        vall[:, bb * KC + jl * 4 + jj, :, 0:HD],
                        vtr_ps[:, jj * 64:(jj + 1) * 64])

        def emit_attention(bb, hh):
            if True:
                qrow = 32 * hh            # q rows in qkr
                krow = 32 * hh            # k rows in ka
                ppks = []
                pavs = {}

                def av_column(qc):
                    # av column for qc (P rows kc<=qc all exist);
                    # 8 query chunks per PSUM bank, normalized per group
                    g = qc // 8
                    if qc % 8 == 0:
                        pavs[g] = tile_c()
                    slot = pavs[g][:, qc % 8, :]
                    for kp in range(qc + 1):
                        nc.tensor.matmul(
                            slot,
                            ppks[kp][:, 128 * (qc - kp):128 * (qc - kp) + 128],
                            vall[:, bb * KC + kp, hh, :],
                            start=(kp == 0), stop=(kp == qc),
                        )
                    if qc % 8 == 7:
                        # normalize this group: attnout = av / l
                        pav = pavs[g]
                        rl = spool.tile([128, 8, 1], F32, tag="rl")
                        nc.vector.reciprocal(rl, pav[:, :, HD:HD + 1])
                        nc.vector.tensor_mul(
                            ao[:, bb, g * 8:(g + 1) * 8, hh, :],
                            pav[:, :, 0:HD],
                            _bcast_free(rl[:, :, 0], HD),
                        )

                # av columns trail the score/exp stream by 2 key chunks so
                # the PE never stalls waiting for the exp it just queued
                for kc in range(KC + 2):
                    if kc < KC:
                        n_kc = T - 128 * kc
                        # narrower first tile on the very first pair so the
                        # first exp fires one proj chunk earlier
                        cw = 1024
                        kslc = slice(bb * T + 128 * kc, bb * T + 128 * (kc + 1))
                        # per-kc P tile: precise deps (av reads never block
                        # later exps) and half the packed-tile footprint
                        ppk = ppool.tile([128, n_kc], BF16, tag=f"ppk{kc}",
                                         bufs=(2 if kc < 5 else 1),
                                         name=f"ppk{kc}")
                        ppks.append(ppk)
                        for c0 in range(0, n_kc, cw):
                            nt = min(cw, n_kc - c0)
                            sc_ps = tile_a()
                            for c in range(c0, c0 + nt, 512):
                                ln = min(512, n_kc - c)
                                qslc = slice(bb * T + 128 * kc + c,
                                             bb * T + 128 * kc + c + ln)
                                nc.tensor.matmul(
                                    sc_ps[:, c - c0:c - c0 + ln],
                                    ka[krow:krow + 32, kslc],
                                    qkr[qrow:qrow + 32, qslc],
                                    start=True, stop=(c > 0),
                                    skip_group_check=True,
                                )
                            if c0 == 0:
                                # causal mask on the diagonal 128x128 block:
                                # accumulates -240*max(0, k-q)
                                nc.tensor.matmul(
                                    sc_ps[:, 0:128], trilA, trilB,
                                    start=False, stop=True,
                                    skip_group_check=True,
                                )
                            nc.scalar.activation(
                                out=ppk[:, c0:c0 + nt],
                                in_=sc_ps[:, 0:nt],
                                func=EXP, scale=SCALE,
                            )
                    if kc >= 2:
                        av_column(kc - 2)

        def emit_epilogue(bb, last):
            # transpose attnout group g, then immediately out-proj its 4
            # row chunks so the tail drains incrementally
            for g in range(4):
                at_ps = psum.tile([64, 512], BF16, tag="B", bufs=2,
                                  name="psBt")
                for jj in range(4):
                    qc = g * 4 + jj
                    nc.tensor.transpose(
                        at_ps[:, jj * 128:(jj + 1) * 128],
                        ao[:, bb, qc, :, :].rearrange("p a b -> p (a b)"),
                        identb,
                    )
                if last and g % 2 == 1:
                    nc.scalar.copy(
                        aoT[:, bb * T + g * 512:bb * T + (g + 1) * 512],
                        at_ps)
                else:
                    nc.vector.tensor_copy(
                        aoT[:, bb * T + g * 512:bb * T + (g + 1) * 512],
                        at_ps)
                for qc in range(g * 4, g * 4 + 4):
                    rc = bb * KC + qc
                    out_ps = tile_b()
                    nc.tensor.matmul(
                        out_ps, aoT[:, rc * 128:(rc + 1) * 128],
                        wo, start=True, stop=True,
                    )
                    out_sb = spool.tile([128, 512], F32, tag="outsb", bufs=8)
                    if last and qc % 2 == 1:
                        nc.scalar.copy(out_sb, out_ps)
                    else:
                        nc.vector.tensor_copy(out_sb, out_ps)
                    eng = nc.sync if qc % 2 == 0 else nc.gpsimd
                    eng.dma_start(
                        out=io["out_part"][rc * 128:(rc + 1) * 128, :],
                        in_=out_sb,
                    )

        # software-pipelined emission: later batches' proj and earlier
        # batches' epilogues fill engine gaps in the exp-paced attention
        for _it in range(loop_k):
            emit_proj(0)
            emit_attention(0, 0)
            emit_attention(0, 1)
            emit_proj(1)
            emit_attention(1, 0)
            emit_epilogue(0, last=False)
            emit_attention(1, 1)
            emit_epilogue(1, last=True)


def build_program(loop_k=1):
    nc = bacc.Bacc(
        "TRN2", target_bir_lowering=False, debug=False,
        enable_asserts=True, num_devices=NCORES,
    )
    io = {}
    for name, shape, dt_ in [
        ("xT", [D, R], F32R), ("wqkvT", [D, 192], F32R),
        ("woT", [64, D], BF16),
        ("cosw", [128, T], F32), ("sinw", [128, T], F32),
        ("consts1", [128, 384], F32R), ("ident", [128, 128], F32),
    ]:
        io[name] = nc.dram_tensor(name, shape, dt_, kind="ExternalInput").ap()
    io["out_part"] = nc.dram_tensor("out_part", [R, D], F32,
                                    kind="ExternalOutput").ap()
    with tile.TileContext(nc) as tc:
        _emit(tc, io, loop_k=loop_k)
    nc.compile()
    return nc


def host_constants():
    t = np.arange(T, dtype=np.float32)
    inv_freq = (1.0 / (10000.0 ** (np.arange(0, HD, 2, dtype=np.float32) / HD)))
    freqs = np.outer(t, inv_freq).astype(np.float32)      # (T, 16)
    emb = np.concatenate([freqs, freqs], axis=-1)         # (T, 32)
    cos = np.cos(emb).astype(np.float32)
    sin = np.sin(emb).astype(np.float32)
    cosw = np.tile(cos.T, (4, 1)).astype(np.float32)      # (128, 2048)
    ssin = sin.T.copy()
    ssin[:HD // 2] *= -1.0                                # signed sin
    sinw = np.tile(ssin, (4, 1)).astype(np.float32)

    permt = np.zeros((128, 128), dtype=np.float32)
    for blk in range(4):
        for m in range(HD):
            permt[blk * HD + (m + HD // 2) % HD, blk * HD + m] = 1.0

    a = np.arange(128)
    trilA = np.where(a[:, None] <= a[None, :], MASK_VAL, 0.0).astype(np.float32)
    trilB = np.where(a[:, None] > a[None, :], 1.0, 0.0).astype(np.float32)
    ident = np.eye(128, dtype=np.float32)
    consts1 = np.concatenate([permt, trilA, trilB], axis=1)
    return dict(cosw=cosw, sinw=sinw, ident=ident,
                consts1=np.ascontiguousarray(consts1))


def core_inputs(x, w_qkv, w_o):
    """Per-core input maps (core c owns heads 2c, 2c+1)."""
    x = np.asarray(x, dtype=np.float32)
    w_qkv = np.asarray(w_qkv, dtype=np.float32)
    w_o = np.asarray(w_o, dtype=np.float32)
    xT = np.ascontiguousarray(x.reshape(R, D).T)
    consts = host_constants()
    maps = []
    for c in range(NCORES):
        h0 = NHL * c
        qrows = w_qkv[h0 * HD:(h0 + NHL) * HD]                  # (64, 512)
        krows = w_qkv[D + h0 * HD:D + (h0 + NHL) * HD]
        vrows = w_qkv[2 * D + h0 * HD:2 * D + (h0 + NHL) * HD]
        m = dict(consts)
        m["xT"] = xT
        m["wqkvT"] = np.ascontiguousarray(
            np.concatenate([qrows, krows, vrows], axis=0).T)     # (512, 192)
        m["woT"] = np.ascontiguousarray(
            w_o[:, h0 * HD:(h0 + NHL) * HD].T).astype(ml_dtypes.bfloat16)
        maps.append(m)
    return maps


_PROG = None


def _get_prog():
    global _PROG
    if _PROG is None:
        _PROG = build_program()
    return _PROG


def kernel(x, w_qkv, w_o):
    nc = _get_prog()
    maps = core_inputs(x, w_qkv, w_o)
    res = run_bass_kernel_spmd(nc, maps, list(range(NCORES)))
    acc = np.zeros((R, D), dtype=np.float32)
    for i in range(NCORES):
        acc += res.results[i]["out_part"]
    return acc.reshape(B, T, D)



# revision 3
# speedup vs baseline: 2.9394x; 2.9394x over previous
"""Trainium2 Bass kernel for MultiHeadSelfAttention (RoPE + causal softmax).

Problem (hardcoded):
  x: (2, 2048, 512) f32, w_qkv: (1536, 512) f32, w_o: (512, 512) f32
  D_MODEL=512, N_HEADS=16, HEAD_DIM=32, ROPE_BASE=10000, causal.

Sharding: tensor-parallel over heads. Core c owns heads (2c, 2c+1) for both
batches; host sums the 8 row-parallel out-projection partials (bf16).

Engine plan (per core):
  - all matmul operands bf16 (1 cycle/row at any width)
  - scores S.T run as 4 concurrent PE row-tiles (K=32): strips are
    (head, kc-parity) at row offsets 0/32/64/96; q and k replicated to the
    upper partition half by gpsimd +-64 partition-shift copies.
  - softmax exp split between ScalarE (table exp) and VectorE via a
    single-op Schraudolph trick: u16 = sat(round(s*A + B)) bitcast to bf16.
  - AV runs V-stationary as 4 concurrent PE col-tiles {av_h0, av_h1, l_h0,
    l_h1} (M=32 each) producing the transposed attention output plus
    replicated softmax row-sums in one PSUM bank; no epilogue transposes.
  - normalize: reciprocal_approx_fast (1 DVE op) + gpsimd -64 shift +
    one DVE multiply into aoT (bf16).
  - out-projection aoT.T @ woT, PSUM->SBUF bf16 copy (ScalarE/VectorE
    alternating), bf16 DMA out.
"""

import sys
import math
from contextlib import ExitStack

sys.path.insert(0, "/opt/trn_rl_repo")

import numpy as np
import ml_dtypes

import concourse.bass as bass
import concourse.tile as tile
from concourse import bacc, mybir
from concourse.bass_utils import run_bass_kernel_spmd

F32 = mybir.dt.float32
BF16 = mybir.dt.bfloat16
U16 = mybir.dt.uint16
EXP = mybir.ActivationFunctionType.Exp
MULT = mybir.AluOpType.mult
ADD = mybir.AluOpType.add

B = 2
T = 2048
D = 512
NH = 16
HD = 32
NCORES = 8
R = B * T
NHL = NH // NCORES   # 2 heads per core
KC = T // 128        # 16 key chunks of 128 per batch
NJ = T // 512        # 4 query chunks of 512 per batch
SCALE = 1.0 / math.sqrt(HD)
MASK_VAL = -240.0
LOG2E = 1.4426950408889634
A16 = 128.0 * LOG2E * SCALE
B16 = 127.0 * 128.0 - 5.5

# of every 8 exp chunks, this many go to the DVE (Schraudolph); rest to ACT
DVE_EXP_N8 = 0


def _emit(tc, io, loop_k=1):
    nc = tc.nc
    with ExitStack() as ctx:
        cpool = ctx.enter_context(tc.tile_pool(name="consts", bufs=1))
        mpool = ctx.enter_context(tc.tile_pool(name="main", bufs=1))
        spool = ctx.enter_context(tc.tile_pool(name="small", bufs=3))
        opool = ctx.enter_context(tc.tile_pool(name="outs", bufs=8))
        ppool = ctx.enter_context(tc.tile_pool(name="pk", bufs=1))
        # PSUM (8 banks): S [128,4,512] = 4 banks, AVO tag x2 = 2 banks,
        # P1 (qk/sh serial) = 1 bank, P2 (v/vtr serial) = 1 bank.
        psum = ctx.enter_context(tc.tile_pool(name="psum", bufs=1, space="PSUM"))

        # ---- constants ----
        cmix = cpool.tile([128, 384], BF16, tag="cmix")
        nc.gpsimd.dma_start(out=cmix, in_=io["cmix"])
        permt = cmix[:, 0:128]
        trilA = cmix[:, 128:256]
        trilB = cmix[:, 256:384]
        identb = cpool.tile([64, 64], BF16, tag="identb")
        nc.gpsimd.dma_start(out=identb, in_=io["identb"])
        wo = cpool.tile([64, 512], BF16, tag="wo")
        nc.scalar.dma_start(out=wo, in_=io["woT"])
        cosw = cpool.tile([128, T], F32, tag="cosw")
        sinw = cpool.tile([128, T], F32, tag="sinw")
        nc.scalar.dma_start(out=cosw, in_=io["cosw"])
        nc.scalar.dma_start(out=sinw, in_=io["sinw"])
        wqkv = []
        for dc in range(4):
            w_t = cpool.tile([128, 192], BF16, tag=f"wqkv{dc}", name=f"wq{dc}")
            nc.gpsimd.dma_start(out=w_t, in_=io["wqkvT"][dc * 128:(dc + 1) * 128, :])
            wqkv.append(w_t)
        ones32 = cpool.tile([128, 32], BF16, tag="ones32")
        nc.vector.memset(ones32, 1.0)

        # ---- per-batch persistent activations ----
        # qkrope: rows 0-63 = RoPE'd q (h0,h1), rows 64-127 = RoPE'd k
        qkrope = [mpool.tile([128, T], BF16, tag=f"qkr{b}", name=f"qkr{b}")
                  for b in range(B)]
        # qhi: rows 64-127 = copy of q (for odd-parity score strips)
        qhi = [mpool.tile([128, T], BF16, tag=f"qhi{b}", name=f"qhi{b}")
               for b in range(B)]
        # kev: rows 0-63 = k shifted down 64, for even kc (slot kc//2)
        kev = [mpool.tile([64, KC // 2, 128], BF16, tag=f"kev{b}", name=f"kev{b}")
               for b in range(B)]
        vall = [mpool.tile([128, KC, NHL, HD], BF16, tag=f"vall{b}", name=f"va{b}")
                for b in range(B)]
        aoT = [mpool.tile([64, T], BF16, tag=f"aoT{b}", name=f"aoT{b}")
               for b in range(B)]
        rl = mpool.tile([128, 512], F32, tag="rl")

        expctr = [0]

        def emit_proj(b):
            xt = [mpool.tile([128, T], BF16, tag=f"xt{dc}", bufs=1,
                             name=f"xt{dc}") for dc in range(4)]
            for jq in range(4):
                for dc in range(4):
                    nc.sync.dma_start(
                        out=xt[dc][:, jq * 512:(jq + 1) * 512],
                        in_=io["xT"][dc * 128:(dc + 1) * 128,
                                     b * T + jq * 512:b * T + (jq + 1) * 512],
                    )
            for jl in range(4):
                cols = slice(jl * 512, (jl + 1) * 512)
                # qk projection -> P1
                qk_ps = psum.tile([128, 512], F32, tag="P1", bufs=1, name="qkps")
                for dc in range(4):
                    nc.tensor.matmul(qk_ps, wqkv[dc][:, 0:128],
                                     xt[dc][:, cols], start=(dc == 0),
                                     stop=(dc == 3))
                t2 = spool.tile([128, 512], BF16, tag="t2", name="t2")
                nc.vector.tensor_mul(t2, qk_ps, cosw[:, cols])
                qks = spool.tile([128, 512], BF16, tag="qks", name="qks")
                nc.scalar.copy(qks, qk_ps)
                # rotate-half permutation -> reuse P1 bank
                sh_ps = psum.tile([128, 512], F32, tag="P1", bufs=1, name="shps")
                nc.tensor.matmul(sh_ps, permt, qks, start=True, stop=True)
                t1 = spool.tile([128, 512], BF16, tag="t1", name="t1")
                nc.vector.tensor_mul(t1, sh_ps, sinw[:, cols])
                nc.vector.tensor_add(qkrope[b][:, cols], t1, t2)
                # v projection -> P2
                v_ps = psum.tile([64, 512], F32, tag="P2", bufs=1, name="vps")
                for dc in range(4):
                    nc.tensor.matmul(v_ps, wqkv[dc][:, 128:192],
                                     xt[dc][:, cols], start=(dc == 0),
                                     stop=(dc == 3))
                v_sb = spool.tile([64, 512], BF16, tag="vsb", name="vsb")
                nc.scalar.copy(v_sb, v_ps)
                vtr_ps = psum.tile([128, 4, 2, HD], BF16, tag="P2", bufs=1,
                                   name="vtrps")
                for jj in range(4):
                    nc.tensor.transpose(
                        vtr_ps[:, jj, :, :].rearrange("p a b -> p (a b)"),
                        v_sb[:, jj * 128:(jj + 1) * 128], identb)
                nc.vector.tensor_copy(vall[b][:, jl * 4:jl * 4 + 4, :, :], vtr_ps)
                # replicas for the score strips
                nc.gpsimd.tensor_copy(qhi[b][64:128, cols], qkrope[b][0:64, cols])
                for kc in (4 * jl, 4 * jl + 2):
                    nc.gpsimd.tensor_copy(
                        kev[b][0:64, kc // 2, :],
                        qkrope[b][64:128, kc * 128:(kc + 1) * 128])

        def emit_attn_j(b, j):
            nkc = 4 * j + 4
            ppk = ppool.tile([128, nkc, NHL, 512], BF16, tag=f"ppk{j}", bufs=1,
                             name=f"ppk{j}")
            S = psum.tile([128, 4, 512], F32, tag="S", bufs=1, name="S")
            qlo = slice(j * 512, (j + 1) * 512)

            # ---- scores + exp, per key chunk ----
            for kc in range(nkc):
                par = kc & 1
                c0 = max(0, 128 * kc - 512 * j)
                nw = 512 - c0
                qcols = slice(512 * j + c0, 512 * (j + 1))
                diag = kc >= 4 * j
                for h in range(NHL):
                    roff = 64 * par + 32 * h
                    sidx = 2 * par + h
                    if par == 0:
                        ksrc = kev[b][32 * h:32 * h + 32, kc // 2, :]
                        qsrc = qkrope[b][roff:roff + 32, qcols]
                    else:
                        ksrc = qkrope[b][roff:roff + 32,
                                         kc * 128:(kc + 1) * 128]
                        qsrc = qhi[b][roff:roff + 32, qcols]
                    nc.tensor.matmul(
                        S[:, sidx, c0:512], ksrc, qsrc,
                        start=True, stop=not diag,
                        tile_position=(roff, 0), skip_group_check=True)
                    if diag:
                        nc.tensor.matmul(
                            S[:, sidx, c0:c0 + 128], trilA, trilB,
                            start=False, stop=True,
                            tile_position=(0, 0), skip_group_check=True)
                # exp over both heads' strips of this kc
                use_dve = (expctr[0] % 8) < DVE_EXP_N8
                expctr[0] += 1
                if use_dve:
                    nc.vector.tensor_scalar(
                        out=ppk[:, kc, :, c0:512].bitcast(U16),
                        in0=S[:, 2 * par:2 * par + 2, c0:512],
                        scalar1=A16, scalar2=B16, op0=MULT, op1=ADD)
                else:
                    nc.scalar.activation(
                        out=ppk[:, kc, :, c0:512],
                        in_=S[:, 2 * par:2 * par + 2, c0:512],
                        func=EXP, scale=SCALE)

            # ---- AV: 4 col-tiles {av0, av1, l0, l1} ----
            av = psum.tile([128, 512], F32, tag="AVO", bufs=2, name="av")
            for kp in range(nkc):
                c0 = max(0, 128 * kp - 512 * j)
                srcs = (vall[b][:, kp, 0, :], vall[b][:, kp, 1, :],
                        ones32, ones32)
                for g in range(4):
                    nc.tensor.matmul(
                        av[32 * g:32 * g + 32, c0:512], srcs[g],
                        ppk[:, kp, g & 1, c0:512],
                        start=(kp == 0), stop=(kp == nkc - 1),
                        tile_position=(0, 32 * g), skip_group_check=True)

            # ---- normalize into aoT ----
            # full-tile: custom DVE ops misbehave on partition-offset slices;
            # rows 0-63 are garbage reciprocals, overwritten by the shift below
            nc.vector.reciprocal_approx_fast(out=rl, in_=av)
            nc.gpsimd.tensor_copy(rl[0:64, :], rl[64:128, :])
            nc.vector.tensor_mul(aoT[b][:, qlo], av[0:64, :], rl[0:64, :])

            # ---- out-projection + DMA ----
            for qb in range(4):
                o_ps = psum.tile([128, 512], F32, tag="AVO", bufs=2, name="ops")
                nc.tensor.matmul(
                    o_ps, aoT[b][:, 512 * j + 128 * qb:512 * j + 128 * (qb + 1)],
                    wo, start=True, stop=True)
                o_sb = opool.tile([128, 512], BF16, tag="osb", name="osb")
                if qb % 2 == 0:
                    nc.scalar.copy(o_sb, o_ps)
                else:
                    nc.vector.tensor_copy(o_sb, o_ps)
                rc = b * T + 512 * j + 128 * qb
                nc.sync.dma_start(out=io["out_part"][rc:rc + 128, :], in_=o_sb)

        for _it in range(loop_k):
            emit_proj(0)
            emit_attn_j(0, 0)
            emit_attn_j(0, 1)
            emit_proj(1)
            emit_attn_j(0, 2)
            emit_attn_j(0, 3)
            for j in range(4):
                emit_attn_j(1, j)


def build_program(loop_k=1):
    nc = bacc.Bacc(
        "TRN2", target_bir_lowering=False, debug=False,
        enable_asserts=True, num_devices=NCORES,
    )
    io = {}
    for name, shape, dt_ in [
        ("xT", [D, R], BF16), ("wqkvT", [D, 192], BF16),
        ("woT", [64, D], BF16),
        ("cosw", [128, T], F32), ("sinw", [128, T], F32),
        ("cmix", [128, 384], BF16), ("identb", [64, 64], BF16),
    ]:
        io[name] = nc.dram_tensor(name, shape, dt_, kind="ExternalInput").ap()
    io["out_part"] = nc.dram_tensor("out_part", [R, D], BF16,
                                    kind="ExternalOutput").ap()
    with tile.TileContext(nc) as tc:
        _emit(tc, io, loop_k=loop_k)
    nc.compile()
    return nc


def host_constants():
    t = np.arange(T, dtype=np.float32)
    inv_freq = (1.0 / (10000.0 ** (np.arange(0, HD, 2, dtype=np.float32) / HD)))
    freqs = np.outer(t, inv_freq).astype(np.float32)      # (T, 16)
    emb = np.concatenate([freqs, freqs], axis=-1)         # (T, 32)
    cos = np.cos(emb).astype(np.float32)
    sin = np.sin(emb).astype(np.float32)
    cosw = np.tile(cos.T, (4, 1)).astype(np.float32)      # (128, 2048)
    ssin = sin.T.copy()
    ssin[:HD // 2] *= -1.0                                # signed sin
    sinw = np.tile(ssin, (4, 1)).astype(np.float32)

    permt = np.zeros((128, 128), dtype=np.float32)
    for blk in range(4):
        for m in range(HD):
            permt[blk * HD + (m + HD // 2) % HD, blk * HD + m] = 1.0

    a = np.arange(128)
    trilA = np.where(a[:, None] <= a[None, :], MASK_VAL, 0.0).astype(np.float32)
    trilB = np.where(a[:, None] > a[None, :], 1.0, 0.0).astype(np.float32)
    cmix = np.concatenate([permt, trilA, trilB], axis=1)
    bf = ml_dtypes.bfloat16
    return dict(
        cosw=cosw, sinw=sinw,
        cmix=np.ascontiguousarray(cmix).astype(bf),
        identb=np.eye(64, dtype=np.float32).astype(bf),
    )


def core_inputs(x, w_qkv, w_o):
    """Per-core input maps (core c owns heads 2c, 2c+1)."""
    bf = ml_dtypes.bfloat16
    x = np.asarray(x, dtype=np.float32)
    w_qkv = np.asarray(w_qkv, dtype=np.float32)
    w_o = np.asarray(w_o, dtype=np.float32)
    xT = np.ascontiguousarray(x.reshape(R, D).T).astype(bf)
    consts = host_constants()
    maps = []
    for c in range(NCORES):
        h0 = NHL * c
        qrows = w_qkv[h0 * HD:(h0 + NHL) * HD]                  # (64, 512)
        krows = w_qkv[D + h0 * HD:D + (h0 + NHL) * HD]
        vrows = w_qkv[2 * D + h0 * HD:2 * D + (h0 + NHL) * HD]
        m = dict(consts)
        m["xT"] = xT
        m["wqkvT"] = np.ascontiguousarray(
            np.concatenate([qrows, krows, vrows], axis=0).T).astype(bf)
        m["woT"] = np.ascontiguousarray(
            w_o[:, h0 * HD:(h0 + NHL) * HD].T).astype(bf)
        maps.append(m)
    return maps


_PROG = None


def _get_prog():
    global _PROG
    if _PROG is None:
        _PROG = build_program()
    return _PROG


def kernel(x, w_qkv, w_o):
    nc = _get_prog()
    maps = core_inputs(x, w_qkv, w_o)
    res = run_bass_kernel_spmd(nc, maps, list(range(NCORES)))
    acc = np.zeros((R, D), dtype=np.float32)
    for i in range(NCORES):
        acc += res.results[i]["out_part"].astype(np.float32)
    return acc.reshape(B, T, D)
